# revision 7
# baseline (speedup 1.0000x reference)
"""Trainium2 Bass kernel for nn_ExternalMemory (scatter_memory).

Computes, for a KV external-memory module:
  - RoPE on the incoming key segment (Llama convention)
  - full-buffer path: shift keys/values left by one segment, write the
    new (rotated) key segment and value segment at the end
  - non-full path: slotted in-place write at segment `current_memory`

Sharding: tensor-parallel over the 16 heads -> 2 heads per NeuronCore on
8 cores.  All the work is head-independent, so no collectives.

The bulk of the work is pure memory movement, done as large DRAM->DRAM
DMAs.  Only the new key segment goes through SBUF (for the RoPE
multiply-adds on the vector engine).  The RoPE operands (u per head,
cos, sin) are packed host-side into one tensor so a single DMA (and a
single completion semaphore) covers every vector-engine dependency —
DVE instructions only have one sync-wait slot.
"""

import numpy as np

N_CORES = 8
B = 1
H = 16
HPC = H // N_CORES       # heads per core = 2
SEG = 2048               # segment length
MEM = 8                  # number of memory slots
TOTAL = MEM * SEG        # 16384
D = 128                  # head dim
HALF = D // 2
PB = 128                 # SBUF partitions
NB = SEG // PB           # position blocks per segment = 16
RJ = HPC + 2             # packed rope rows: u[0], u[1], cos, sin

_prog_cache: dict = {}


def _split_multi_waits(nc, mybir):
    """Walrus codegen only allows ONE sync-wait per instruction; Tile's tail
    drain can carry several (one per outstanding DMA sem lane).  Split any
    multi-wait instruction into a chain of single-wait no-ops on the same
    engine (semantics preserved: the engine blocks at the no-ops instead)."""
    for fn in nc.m.functions:
        for bb in fn.blocks:
            insts = list(bb.instructions)
            out = []
            n_new = 0
            for inst in insts:
                si = inst.sync_info
                waits = list(si.on_wait) if (si is not None and si.on_wait) else []
                if len(waits) > 1:
                    for j, w in enumerate(waits[:-1]):
                        out.append(mybir.InstNoOp(
                            name=f"{inst.name}_wsplit{j}",
                            engine=inst.engine,
                            bass_nofuse=True,
                            sync_info=mybir.SyncInfo(on_wait=[w], on_update=[]),
                        ))
                        n_new += 1
                    inst.sync_info = mybir.SyncInfo(
                        on_wait=[waits[-1]],
                        on_update=list(si.on_update or []),
                    )
                out.append(inst)
            if n_new:
                bb.instructions = out


def _build_program(write_seg: int, full_shift: bool):
    """Build the per-core Bass program.

    write_seg: segment index where the new K/V segment lands.
    full_shift: True -> shift everything left one segment first;
                False -> copy all segments except write_seg unchanged.
    """
    import concourse.bass as bass
    import concourse.tile as tile
    from concourse import mybir

    f32 = mybir.dt.float32
    nc = bass.Bass(trn_type="TRN2", name="scatter_memory")

    keys = nc.dram_tensor("keys", [HPC, TOTAL, D], f32, kind="ExternalInput")
    values = nc.dram_tensor("values", [HPC, TOTAL, D], f32, kind="ExternalInput")
    # rope_in rows: [u(head 0), u(head 1), cos, sin'], sin' has its first
    # half negated so RoPE is mul/mul/add with no sign handling on-device.
    rope_in = nc.dram_tensor("rope_in", [RJ, SEG, D], f32, kind="ExternalInput")
    v = nc.dram_tensor("v", [HPC, SEG, D], f32, kind="ExternalInput")
    out = nc.dram_tensor("out", [2, HPC, TOTAL, D], f32, kind="ExternalOutput")

    ws = write_seg

    with tile.TileContext(nc) as tc:
        with (
            tc.tile_pool(name="const", bufs=1) as const_pool,
            tc.tile_pool(name="work", bufs=2) as work_pool,
        ):
            # --- RoPE path (through SBUF) on the ACT HWDGE ring ---
            rope_t = const_pool.tile([PB, RJ, NB, D], f32)
            nc.scalar.dma_start(
                out=rope_t[:],
                in_=rope_in[:].rearrange("j (n p) d -> p j n d", p=PB),
            )
            cos_t = rope_t[:, HPC]
            sin_t = rope_t[:, HPC + 1]
            k_t = work_pool.tile([PB, HPC, NB, D], f32, tag="k")
            t_t = work_pool.tile([PB, HPC, NB, D], f32, tag="t")
            for h in range(HPC):
                u_t = rope_t[:, h]
                # t = u * cos
                nc.vector.tensor_mul(t_t[:, h], u_t, cos_t)
                # k[.., :HALF]  = u2 * (-sin1)   (sign folded into sin input)
                nc.vector.tensor_mul(
                    k_t[:, h, :, 0:HALF], u_t[:, :, HALF:D], sin_t[:, :, 0:HALF]
                )
                # k[.., HALF:] = u1 * sin2
                nc.vector.tensor_mul(
                    k_t[:, h, :, HALF:D], u_t[:, :, 0:HALF], sin_t[:, :, HALF:D]
                )
                # k += t
                nc.vector.tensor_add(k_t[:, h], k_t[:, h], t_t[:, h])
                # per-head store (a combined 4-free-dim AP can't be balanced)
                nc.scalar.dma_start(
                    out=out[0, h, ws * SEG:(ws + 1) * SEG, :].rearrange(
                        "(n p) d -> p n d", p=PB
                    ),
                    in_=k_t[:, h],
                )

            # --- bulk copies (DRAM->DRAM) on the SP HWDGE ring ---
            # new value segments into slot ws for both heads (pure copy)
            nc.sync.dma_start(
                out=out[1, :, ws * SEG:(ws + 1) * SEG, :].rearrange("h a b -> h (a b)"),
                in_=v[:].rearrange("h a b -> h (a b)"),
            )
            if full_shift:
                nc.sync.dma_start(
                    out=out[0, :, 0:TOTAL - SEG, :].rearrange("h a b -> h (a b)"),
                    in_=keys[:, SEG:TOTAL, :].rearrange("h a b -> h (a b)"),
                )
                nc.sync.dma_start(
                    out=out[1, :, 0:TOTAL - SEG, :].rearrange("h a b -> h (a b)"),
                    in_=values[:, SEG:TOTAL, :].rearrange("h a b -> h (a b)"),
                )
            else:
                if ws > 0:
                    nc.sync.dma_start(
                        out=out[0, :, 0:ws * SEG, :].rearrange("h a b -> h (a b)"),
                        in_=keys[:, 0:ws * SEG, :].rearrange("h a b -> h (a b)"),
                    )
                    nc.sync.dma_start(
                        out=out[1, :, 0:ws * SEG, :].rearrange("h a b -> h (a b)"),
                        in_=values[:, 0:ws * SEG, :].rearrange("h a b -> h (a b)"),
                    )
                if ws < MEM - 1:
                    nc.sync.dma_start(
                        out=out[0, :, (ws + 1) * SEG:TOTAL, :].rearrange("h a b -> h (a b)"),
                        in_=keys[:, (ws + 1) * SEG:TOTAL, :].rearrange("h a b -> h (a b)"),
                    )
                    nc.sync.dma_start(
                        out=out[1, :, (ws + 1) * SEG:TOTAL, :].rearrange("h a b -> h (a b)"),
                        in_=values[:, (ws + 1) * SEG:TOTAL, :].rearrange("h a b -> h (a b)"),
                    )
    _split_multi_waits(nc, mybir)
    return nc


# Results of the most recent device run (for the test harness to inspect).
LAST_RESULTS = None


def _pack_rope(un_rotated_k_core, cos_seg, sin_mod):
    """[u(h0), u(h1), cos, sin'] -> [RJ, SEG, D] float32 contiguous."""
    packed = np.empty((RJ, SEG, D), dtype=np.float32)
    packed[:HPC] = un_rotated_k_core
    packed[HPC] = cos_seg
    packed[HPC + 1] = sin_mod
    return packed


def kernel(keys, values, un_rotated_k, v, cos_cache, sin_cache,
           position_ids, current_memory):
    from concourse.bass_utils import run_bass_kernel_spmd

    global LAST_RESULTS

    keys = np.asarray(keys, dtype=np.float32)
    values = np.asarray(values, dtype=np.float32)
    un_rotated_k = np.asarray(un_rotated_k, dtype=np.float32)
    v = np.asarray(v, dtype=np.float32)
    cos_cache = np.asarray(cos_cache, dtype=np.float32)
    sin_cache = np.asarray(sin_cache, dtype=np.float32)
    position_ids = np.asarray(position_ids)
    cm = int(current_memory)

    assert keys.shape == (B, H, TOTAL, D), keys.shape
    assert un_rotated_k.shape == (B, H, SEG, D), un_rotated_k.shape

    # Gather the RoPE tables for this segment's positions and fold the
    # rotate_half sign into sin (first half negated).
    pos = position_ids.reshape(-1)
    cos_seg = cos_cache[pos]
    sin_seg = sin_cache[pos]
    sin_mod = np.concatenate([-sin_seg[:, :HALF], sin_seg[:, HALF:]], axis=1)

    full_shift = cm >= MEM
    write_seg = MEM - 1 if full_shift else cm
    key = (write_seg, full_shift)
    if key not in _prog_cache:
        _prog_cache[key] = _build_program(write_seg, full_shift)
    nc = _prog_cache[key]

    in_maps = []
    for c in range(N_CORES):
        h0 = c * HPC
        in_maps.append({
            "keys": np.ascontiguousarray(keys[0, h0:h0 + HPC]),
            "values": np.ascontiguousarray(values[0, h0:h0 + HPC]),
            "rope_in": _pack_rope(un_rotated_k[0, h0:h0 + HPC], cos_seg, sin_mod),
            "v": np.ascontiguousarray(v[0, h0:h0 + HPC]),
        })

    res = run_bass_kernel_spmd(nc, in_maps, core_ids=list(range(N_CORES)))
    LAST_RESULTS = res

    full = np.empty((2, B, H, TOTAL, D), dtype=np.float32)
    for c in range(N_CORES):
        h0 = c * HPC
        full[0, 0, h0:h0 + HPC] = res.results[c]["out"][0]
        full[1, 0, h0:h0 + HPC] = res.results[c]["out"][1]
    return full


# revision 9
# speedup vs baseline: 5.8880x; 5.8880x over previous
"""Trainium2 Bass kernel for nn_ExternalMemory (scatter_memory).

Computes, for a KV external-memory module:
  - RoPE on the incoming key segment (Llama convention)
  - full-buffer path: shift keys/values left by one segment, write the
    new (rotated) key segment and value segment at the end
  - non-full path: slotted in-place write at segment `current_memory`

Sharding: tensor-parallel over the 16 heads -> 2 heads per NeuronCore on
8 cores.  All the work is head-independent, so no collectives.

Key performance facts (measured on trn2 via For_i repeat-loop timing):
  - DRAM->DRAM DMA with a flat 1D AP moves ~209 GB/s (one 7 MiB copy);
    a combined [2, N] two-head AP collapses to ~42 GB/s because the
    outer dim caps SDMA engine fan-out.  So every bulk copy here is a
    flat per-head 1D AP, split in 2 chunks.
  - The bulk shift copies go on the SP HWDGE ring (no waits -> the SP
    sequencer never stalls); the RoPE path (load/compute/store) lives on
    the ACT ring + DVE so it overlaps the bulk copies.
  - Walrus codegen allows only ONE sync-wait per instruction; Tile's
    tail drain can carry one wait per outstanding DMA sem lane, so
    `_split_multi_waits` rewrites multi-wait instructions into chains of
    single-wait no-ops.
  - The RoPE operands (u per head, cos, sin') are packed host-side into
    one tensor so a single DMA (= a single completion semaphore) covers
    every vector-engine dependency (again the 1-wait limit).

Per-core HBM traffic: 34 MiB read + 32 MiB write = 66 MiB; measured
~200 us/core against a ~193 us roofline at 358 GB/s.
"""

import numpy as np

N_CORES = 8
B = 1
H = 16
HPC = H // N_CORES       # heads per core = 2
SEG = 2048               # segment length
MEM = 8                  # number of memory slots
TOTAL = MEM * SEG        # 16384
D = 128                  # head dim
HALF = D // 2
PB = 128                 # SBUF partitions
NB = SEG // PB           # position blocks per segment = 16
RJ = HPC + 2             # packed rope rows: u[0], u[1], cos, sin'
CHUNK = 2                # bulk-copy split factor

_prog_cache: dict = {}


def _split_multi_waits(nc, mybir):
    """Walrus codegen only allows ONE sync-wait per instruction; Tile's tail
    drain can carry several (one per outstanding DMA sem lane).  Split any
    multi-wait instruction into a chain of single-wait no-ops on the same
    engine (semantics preserved: the engine blocks at the no-ops instead)."""
    for fn in nc.m.functions:
        for bb in fn.blocks:
            insts = list(bb.instructions)
            out = []
            n_new = 0
            for inst in insts:
                si = inst.sync_info
                waits = list(si.on_wait) if (si is not None and si.on_wait) else []
                if len(waits) > 1:
                    for j, w in enumerate(waits[:-1]):
                        out.append(mybir.InstNoOp(
                            name=f"{inst.name}_wsplit{j}",
                            engine=inst.engine,
                            bass_nofuse=True,
                            sync_info=mybir.SyncInfo(on_wait=[w], on_update=[]),
                        ))
                        n_new += 1
                    inst.sync_info = mybir.SyncInfo(
                        on_wait=[waits[-1]],
                        on_update=list(si.on_update or []),
                    )
                out.append(inst)
            if n_new:
                bb.instructions = out


def emit_body(nc, const_pool, work_pool, rope_in, keys, values, v, out,
              write_seg, full_shift):
    """Emit one full per-core kernel body (RoPE + bulk copies)."""
    import concourse.mybir as mybir
    f32 = mybir.dt.float32
    ws = write_seg

    # --- RoPE path (through SBUF) on the ACT HWDGE ring ---
    rope_t = const_pool.tile([PB, RJ, NB, D], f32, tag="rope")
    nc.scalar.dma_start(
        out=rope_t[:],
        in_=rope_in[:].rearrange("j (n p) d -> p j n d", p=PB),
    )
    cos_t = rope_t[:, HPC]
    sin_t = rope_t[:, HPC + 1]
    k_t = work_pool.tile([PB, HPC, NB, D], f32, tag="k")
    t_t = work_pool.tile([PB, HPC, NB, D], f32, tag="t")
    for h in range(HPC):
        u_t = rope_t[:, h]
        # t = u * cos
        nc.vector.tensor_mul(t_t[:, h], u_t, cos_t)
        # k[.., :HALF]  = u2 * (-sin1)   (sign folded into sin input)
        nc.vector.tensor_mul(
            k_t[:, h, :, 0:HALF], u_t[:, :, HALF:D], sin_t[:, :, 0:HALF]
        )
        # k[.., HALF:] = u1 * sin2
        nc.vector.tensor_mul(
            k_t[:, h, :, HALF:D], u_t[:, :, 0:HALF], sin_t[:, :, HALF:D]
        )
        # k += t
        nc.vector.tensor_add(k_t[:, h], k_t[:, h], t_t[:, h])
        # per-head store (a combined 4-free-dim AP can't be balanced)
        nc.scalar.dma_start(
            out=out[0, h, ws * SEG:(ws + 1) * SEG, :].rearrange(
                "(n p) d -> p n d", p=PB
            ),
            in_=k_t[:, h],
        )

    # --- bulk copies (DRAM->DRAM), flat 1D APs, on the SP HWDGE ring ---
    def flat_copy(kv, h, dst_lo, src, src_lo, npos):
        step = npos // CHUNK if npos % CHUNK == 0 else npos
        nch = npos // step
        for c in range(nch):
            nc.sync.dma_start(
                out=out[kv, h, dst_lo + c * step:dst_lo + (c + 1) * step, :]
                    .rearrange("a b -> (a b)"),
                in_=src[h, src_lo + c * step:src_lo + (c + 1) * step, :]
                    .rearrange("a b -> (a b)"),
            )

    for h in range(HPC):
        # new value segment into slot ws (pure copy)
        flat_copy(1, h, ws * SEG, v, 0, SEG)
        if full_shift:
            flat_copy(0, h, 0, keys, SEG, TOTAL - SEG)
            flat_copy(1, h, 0, values, SEG, TOTAL - SEG)
        else:
            if ws > 0:
                flat_copy(0, h, 0, keys, 0, ws * SEG)
                flat_copy(1, h, 0, values, 0, ws * SEG)
            if ws < MEM - 1:
                flat_copy(0, h, (ws + 1) * SEG, keys, (ws + 1) * SEG,
                          TOTAL - (ws + 1) * SEG)
                flat_copy(1, h, (ws + 1) * SEG, values, (ws + 1) * SEG,
                          TOTAL - (ws + 1) * SEG)


def _build_program(write_seg: int, full_shift: bool):
    """Build the per-core Bass program.

    write_seg: segment index where the new K/V segment lands.
    full_shift: True -> shift everything left one segment first;
                False -> copy all segments except write_seg unchanged.
    """
    import concourse.bass as bass
    import concourse.tile as tile
    from concourse import mybir

    f32 = mybir.dt.float32
    nc = bass.Bass(trn_type="TRN2", name="scatter_memory")

    keys = nc.dram_tensor("keys", [HPC, TOTAL, D], f32, kind="ExternalInput")
    values = nc.dram_tensor("values", [HPC, TOTAL, D], f32, kind="ExternalInput")
    # rope_in rows: [u(head 0), u(head 1), cos, sin'], sin' has its first
    # half negated so RoPE is mul/mul/add with no sign handling on-device.
    rope_in = nc.dram_tensor("rope_in", [RJ, SEG, D], f32, kind="ExternalInput")
    v = nc.dram_tensor("v", [HPC, SEG, D], f32, kind="ExternalInput")
    out = nc.dram_tensor("out", [2, HPC, TOTAL, D], f32, kind="ExternalOutput")

    with tile.TileContext(nc) as tc:
        with (
            tc.tile_pool(name="const", bufs=1) as const_pool,
            tc.tile_pool(name="work", bufs=2) as work_pool,
        ):
            emit_body(nc, const_pool, work_pool, rope_in, keys, values, v, out,
                      write_seg, full_shift)
    _split_multi_waits(nc, mybir)
    return nc


# Results of the most recent device run (for the test harness to inspect).
LAST_RESULTS = None


def _pack_rope(un_rotated_k_core, cos_seg, sin_mod):
    """[u(h0), u(h1), cos, sin'] -> [RJ, SEG, D] float32 contiguous."""
    packed = np.empty((RJ, SEG, D), dtype=np.float32)
    packed[:HPC] = un_rotated_k_core
    packed[HPC] = cos_seg
    packed[HPC + 1] = sin_mod
    return packed


def kernel(keys, values, un_rotated_k, v, cos_cache, sin_cache,
           position_ids, current_memory):
    from concourse.bass_utils import run_bass_kernel_spmd

    global LAST_RESULTS

    keys = np.asarray(keys, dtype=np.float32)
    values = np.asarray(values, dtype=np.float32)
    un_rotated_k = np.asarray(un_rotated_k, dtype=np.float32)
    v = np.asarray(v, dtype=np.float32)
    cos_cache = np.asarray(cos_cache, dtype=np.float32)
    sin_cache = np.asarray(sin_cache, dtype=np.float32)
    position_ids = np.asarray(position_ids)
    cm = int(current_memory)

    assert keys.shape == (B, H, TOTAL, D), keys.shape
    assert un_rotated_k.shape == (B, H, SEG, D), un_rotated_k.shape

    # Gather the RoPE tables for this segment's positions and fold the
    # rotate_half sign into sin (first half negated).
    pos = position_ids.reshape(-1)
    cos_seg = cos_cache[pos]
    sin_seg = sin_cache[pos]
    sin_mod = np.concatenate([-sin_seg[:, :HALF], sin_seg[:, HALF:]], axis=1)

    full_shift = cm >= MEM
    write_seg = MEM - 1 if full_shift else cm
    key = (write_seg, full_shift)
    if key not in _prog_cache:
        _prog_cache[key] = _build_program(write_seg, full_shift)
    nc = _prog_cache[key]

    in_maps = []
    for c in range(N_CORES):
        h0 = c * HPC
        in_maps.append({
            "keys": np.ascontiguousarray(keys[0, h0:h0 + HPC]),
            "values": np.ascontiguousarray(values[0, h0:h0 + HPC]),
            "rope_in": _pack_rope(un_rotated_k[0, h0:h0 + HPC], cos_seg, sin_mod),
            "v": np.ascontiguousarray(v[0, h0:h0 + HPC]),
        })

    res = run_bass_kernel_spmd(nc, in_maps, core_ids=list(range(N_CORES)))
    LAST_RESULTS = res

    full = np.empty((2, B, H, TOTAL, D), dtype=np.float32)
    for c in range(N_CORES):
        h0 = c * HPC
        full[0, 0, h0:h0 + HPC] = res.results[c]["out"][0]
        full[1, 0, h0:h0 + HPC] = res.results[c]["out"][1]
    return full


# revision 10
# speedup vs baseline: 6.9969x; 1.1883x over previous
"""Trainium2 Bass kernel for nn_ExternalMemory (scatter_memory).

Computes, for a KV external-memory module:
  - RoPE on the incoming key segment (Llama convention)
  - full-buffer path: shift keys/values left by one segment, write the
    new (rotated) key segment and value segment at the end
  - non-full path: slotted in-place write at segment `current_memory`

Sharding: tensor-parallel over the 16 heads -> 2 heads per NeuronCore on
8 cores.  All the work is head-independent, so no collectives.

Key performance facts (measured on trn2 via For_i repeat-loop timing):
  - DRAM->DRAM DMA with a flat 1D AP moves ~209 GB/s (one 7 MiB copy);
    a combined [2, N] two-head AP collapses to ~42 GB/s because the
    outer dim caps SDMA engine fan-out.  So every bulk copy here is a
    flat per-head 1D AP, split in 2 chunks.
  - The bulk shift copies go on the SP HWDGE ring (no waits -> the SP
    sequencer never stalls); the RoPE path (load/compute/store) lives on
    the ACT ring + DVE so it overlaps the bulk copies.
  - Walrus codegen allows only ONE sync-wait per instruction; Tile's
    tail drain can carry one wait per outstanding DMA sem lane, so
    `_split_multi_waits` rewrites multi-wait instructions into chains of
    single-wait no-ops.
  - The RoPE operands (u per head, cos, sin') are packed host-side into
    one tensor so a single DMA (= a single completion semaphore) covers
    every vector-engine dependency (again the 1-wait limit).

Per-core HBM traffic: 34 MiB read + 32 MiB write = 66 MiB; measured
~200 us/core against a ~193 us roofline at 358 GB/s.
"""

import numpy as np

N_CORES = 8
B = 1
H = 16
HPC = H // N_CORES       # heads per core = 2
SEG = 2048               # segment length
MEM = 8                  # number of memory slots
TOTAL = MEM * SEG        # 16384
D = 128                  # head dim
HALF = D // 2
PB = 128                 # SBUF partitions
NB = SEG // PB           # position blocks per segment = 16
RJ = HPC + 2             # packed rope rows: u[0], u[1], cos, sin'
CHUNK = 2                # bulk-copy split factor

_prog_cache: dict = {}


def _split_multi_waits(nc, mybir):
    """Walrus codegen only allows ONE sync-wait per instruction; Tile's tail
    drain can carry several (one per outstanding DMA sem lane).  Split any
    multi-wait instruction into a chain of single-wait no-ops on the same
    engine (semantics preserved: the engine blocks at the no-ops instead)."""
    for fn in nc.m.functions:
        for bb in fn.blocks:
            insts = list(bb.instructions)
            out = []
            n_new = 0
            for inst in insts:
                si = inst.sync_info
                waits = list(si.on_wait) if (si is not None and si.on_wait) else []
                if len(waits) > 1:
                    for j, w in enumerate(waits[:-1]):
                        out.append(mybir.InstNoOp(
                            name=f"{inst.name}_wsplit{j}",
                            engine=inst.engine,
                            bass_nofuse=True,
                            sync_info=mybir.SyncInfo(on_wait=[w], on_update=[]),
                        ))
                        n_new += 1
                    inst.sync_info = mybir.SyncInfo(
                        on_wait=[waits[-1]],
                        on_update=list(si.on_update or []),
                    )
                out.append(inst)
            if n_new:
                bb.instructions = out


def emit_body(nc, const_pool, work_pool, rope_in, keys, values, v, out,
              write_seg, full_shift):
    """Emit one full per-core kernel body (RoPE + bulk copies)."""
    import concourse.mybir as mybir
    f32 = mybir.dt.float32
    ws = write_seg

    # --- RoPE path (through SBUF) on the ACT HWDGE ring ---
    rope_t = const_pool.tile([PB, RJ, NB, D], f32, tag="rope")
    nc.scalar.dma_start(
        out=rope_t[:],
        in_=rope_in[:].rearrange("j (n p) d -> p j n d", p=PB),
    )
    cos_t = rope_t[:, HPC]
    sin_t = rope_t[:, HPC + 1]
    k_t = work_pool.tile([PB, HPC, NB, D], f32, tag="k")
    t_t = work_pool.tile([PB, HPC, NB, D], f32, tag="t")
    for h in range(HPC):
        u_t = rope_t[:, h]
        # t = u * cos
        nc.vector.tensor_mul(t_t[:, h], u_t, cos_t)
        # k[.., :HALF]  = u2 * (-sin1)   (sign folded into sin input)
        nc.vector.tensor_mul(
            k_t[:, h, :, 0:HALF], u_t[:, :, HALF:D], sin_t[:, :, 0:HALF]
        )
        # k[.., HALF:] = u1 * sin2
        nc.vector.tensor_mul(
            k_t[:, h, :, HALF:D], u_t[:, :, 0:HALF], sin_t[:, :, HALF:D]
        )
        # k += t
        nc.vector.tensor_add(k_t[:, h], k_t[:, h], t_t[:, h])
        # per-head store (a combined 4-free-dim AP can't be balanced)
        nc.scalar.dma_start(
            out=out[0, h, ws * SEG:(ws + 1) * SEG, :].rearrange(
                "(n p) d -> p n d", p=PB
            ),
            in_=k_t[:, h],
        )

    # --- bulk copies (DRAM->DRAM), flat 1D APs, on the SP HWDGE ring ---
    def flat_copy(kv, h, dst_lo, src, src_lo, npos):
        # chunk only large runs; sub-4MiB transfers lose to per-DMA overhead
        if npos % CHUNK == 0 and npos >= 4 * SEG:
            step = npos // CHUNK
        else:
            step = npos
        nch = npos // step
        for c in range(nch):
            nc.sync.dma_start(
                out=out[kv, h, dst_lo + c * step:dst_lo + (c + 1) * step, :]
                    .rearrange("a b -> (a b)"),
                in_=src[h, src_lo + c * step:src_lo + (c + 1) * step, :]
                    .rearrange("a b -> (a b)"),
            )

    for h in range(HPC):
        # new value segment into slot ws (pure copy)
        flat_copy(1, h, ws * SEG, v, 0, SEG)
        if full_shift:
            flat_copy(0, h, 0, keys, SEG, TOTAL - SEG)
            flat_copy(1, h, 0, values, SEG, TOTAL - SEG)
        else:
            if ws > 0:
                flat_copy(0, h, 0, keys, 0, ws * SEG)
                flat_copy(1, h, 0, values, 0, ws * SEG)
            if ws < MEM - 1:
                flat_copy(0, h, (ws + 1) * SEG, keys, (ws + 1) * SEG,
                          TOTAL - (ws + 1) * SEG)
                flat_copy(1, h, (ws + 1) * SEG, values, (ws + 1) * SEG,
                          TOTAL - (ws + 1) * SEG)


def _build_program(write_seg: int, full_shift: bool):
    """Build the per-core Bass program.

    write_seg: segment index where the new K/V segment lands.
    full_shift: True -> shift everything left one segment first;
                False -> copy all segments except write_seg unchanged.
    """
    import concourse.bass as bass
    import concourse.tile as tile
    from concourse import mybir

    f32 = mybir.dt.float32
    nc = bass.Bass(trn_type="TRN2", name="scatter_memory")

    keys = nc.dram_tensor("keys", [HPC, TOTAL, D], f32, kind="ExternalInput")
    values = nc.dram_tensor("values", [HPC, TOTAL, D], f32, kind="ExternalInput")
    # rope_in rows: [u(head 0), u(head 1), cos, sin'], sin' has its first
    # half negated so RoPE is mul/mul/add with no sign handling on-device.
    rope_in = nc.dram_tensor("rope_in", [RJ, SEG, D], f32, kind="ExternalInput")
    v = nc.dram_tensor("v", [HPC, SEG, D], f32, kind="ExternalInput")
    out = nc.dram_tensor("out", [2, HPC, TOTAL, D], f32, kind="ExternalOutput")

    with tile.TileContext(nc) as tc:
        with (
            tc.tile_pool(name="const", bufs=1) as const_pool,
            tc.tile_pool(name="work", bufs=2) as work_pool,
        ):
            emit_body(nc, const_pool, work_pool, rope_in, keys, values, v, out,
                      write_seg, full_shift)
    _split_multi_waits(nc, mybir)
    return nc


# Results of the most recent device run (for the test harness to inspect).
LAST_RESULTS = None


def _pack_rope(un_rotated_k_core, cos_seg, sin_mod):
    """[u(h0), u(h1), cos, sin'] -> [RJ, SEG, D] float32 contiguous."""
    packed = np.empty((RJ, SEG, D), dtype=np.float32)
    packed[:HPC] = un_rotated_k_core
    packed[HPC] = cos_seg
    packed[HPC + 1] = sin_mod
    return packed


def kernel(keys, values, un_rotated_k, v, cos_cache, sin_cache,
           position_ids, current_memory):
    from concourse.bass_utils import run_bass_kernel_spmd

    global LAST_RESULTS

    keys = np.asarray(keys, dtype=np.float32)
    values = np.asarray(values, dtype=np.float32)
    un_rotated_k = np.asarray(un_rotated_k, dtype=np.float32)
    v = np.asarray(v, dtype=np.float32)
    cos_cache = np.asarray(cos_cache, dtype=np.float32)
    sin_cache = np.asarray(sin_cache, dtype=np.float32)
    position_ids = np.asarray(position_ids)
    cm = int(current_memory)

    assert keys.shape == (B, H, TOTAL, D), keys.shape
    assert un_rotated_k.shape == (B, H, SEG, D), un_rotated_k.shape

    # Gather the RoPE tables for this segment's positions and fold the
    # rotate_half sign into sin (first half negated).
    pos = position_ids.reshape(-1)
    cos_seg = cos_cache[pos]
    sin_seg = sin_cache[pos]
    sin_mod = np.concatenate([-sin_seg[:, :HALF], sin_seg[:, HALF:]], axis=1)

    full_shift = cm >= MEM
    write_seg = MEM - 1 if full_shift else cm
    key = (write_seg, full_shift)
    if key not in _prog_cache:
        _prog_cache[key] = _build_program(write_seg, full_shift)
    nc = _prog_cache[key]

    in_maps = []
    for c in range(N_CORES):
        h0 = c * HPC
        in_maps.append({
            "keys": np.ascontiguousarray(keys[0, h0:h0 + HPC]),
            "values": np.ascontiguousarray(values[0, h0:h0 + HPC]),
            "rope_in": _pack_rope(un_rotated_k[0, h0:h0 + HPC], cos_seg, sin_mod),
            "v": np.ascontiguousarray(v[0, h0:h0 + HPC]),
        })

    res = run_bass_kernel_spmd(nc, in_maps, core_ids=list(range(N_CORES)))
    LAST_RESULTS = res

    full = np.empty((2, B, H, TOTAL, D), dtype=np.float32)
    for c in range(N_CORES):
        h0 = c * HPC
        full[0, 0, h0:h0 + HPC] = res.results[c]["out"][0]
        full[1, 0, h0:h0 + HPC] = res.results[c]["out"][1]
    return full


# revision 12
# speedup vs baseline: 7.6588x; 1.0946x over previous
"""Trainium2 Bass kernel for nn_ExternalMemory (scatter_memory).

Computes, for a KV external-memory module:
  - RoPE on the incoming key segment (Llama convention)
  - full-buffer path: shift keys/values left by one segment, write the
    new (rotated) key segment and value segment at the end
  - non-full path: slotted in-place write at segment `current_memory`

Sharding: tensor-parallel over the 16 heads -> 2 heads per NeuronCore on
8 cores.  All the work is head-independent, so no collectives.

Key performance facts (measured on trn2 via For_i repeat-loop timing):
  - DRAM->DRAM DMA with a flat 1D AP moves ~209 GB/s (one 7 MiB copy);
    a combined [2, N] two-head AP collapses to ~42 GB/s because the
    outer dim caps SDMA engine fan-out.  So every bulk copy here is a
    flat per-head 1D AP, split in 2 chunks.
  - The bulk shift copies go on the SP HWDGE ring (no waits -> the SP
    sequencer never stalls); the RoPE path (load/compute/store) lives on
    the ACT ring + DVE so it overlaps the bulk copies.
  - Walrus codegen allows only ONE sync-wait per instruction; Tile's
    tail drain can carry one wait per outstanding DMA sem lane, so
    `_split_multi_waits` rewrites multi-wait instructions into chains of
    single-wait no-ops.
  - The RoPE operands (u per head, cos, sin') are packed host-side into
    one tensor so a single DMA (= a single completion semaphore) covers
    every vector-engine dependency (again the 1-wait limit).

Per-core HBM traffic: 34 MiB read + 32 MiB write = 66 MiB; measured
~200 us/core against a ~193 us roofline at 358 GB/s.
"""

import numpy as np

N_CORES = 8
B = 1
H = 16
HPC = H // N_CORES       # heads per core = 2
SEG = 2048               # segment length
MEM = 8                  # number of memory slots
TOTAL = MEM * SEG        # 16384
D = 128                  # head dim
HALF = D // 2
PB = 128                 # SBUF partitions
NB = SEG // PB           # position blocks per segment = 16
RJ = HPC + 2             # packed rope rows: u[0], u[1], cos, sin'
CHUNK = 1                # bulk-copy split factor (1 = no split; tested best)

_prog_cache: dict = {}


def _split_multi_waits(nc, mybir):
    """Walrus codegen only allows ONE sync-wait per instruction; Tile's tail
    drain can carry several (one per outstanding DMA sem lane).  Split any
    multi-wait instruction into a chain of single-wait no-ops on the same
    engine (semantics preserved: the engine blocks at the no-ops instead)."""
    for fn in nc.m.functions:
        for bb in fn.blocks:
            insts = list(bb.instructions)
            out = []
            n_new = 0
            for inst in insts:
                si = inst.sync_info
                waits = list(si.on_wait) if (si is not None and si.on_wait) else []
                if len(waits) > 1:
                    for j, w in enumerate(waits[:-1]):
                        out.append(mybir.InstNoOp(
                            name=f"{inst.name}_wsplit{j}",
                            engine=inst.engine,
                            bass_nofuse=True,
                            sync_info=mybir.SyncInfo(on_wait=[w], on_update=[]),
                        ))
                        n_new += 1
                    inst.sync_info = mybir.SyncInfo(
                        on_wait=[waits[-1]],
                        on_update=list(si.on_update or []),
                    )
                out.append(inst)
            if n_new:
                bb.instructions = out


def emit_body(nc, const_pool, work_pool, rope_in, keys, values, v, out,
              write_seg, full_shift):
    """Emit one full per-core kernel body (RoPE + bulk copies)."""
    import concourse.mybir as mybir
    f32 = mybir.dt.float32
    ws = write_seg

    # --- RoPE path (through SBUF) on the ACT HWDGE ring ---
    rope_t = const_pool.tile([PB, RJ, NB, D], f32, tag="rope")
    nc.scalar.dma_start(
        out=rope_t[:],
        in_=rope_in[:].rearrange("j (n p) d -> p j n d", p=PB),
    )
    cos_t = rope_t[:, HPC]
    sin_t = rope_t[:, HPC + 1]
    k_t = work_pool.tile([PB, HPC, NB, D], f32, tag="k")
    t_t = work_pool.tile([PB, HPC, NB, D], f32, tag="t")
    for h in range(HPC):
        u_t = rope_t[:, h]
        # t = u * cos
        nc.vector.tensor_mul(t_t[:, h], u_t, cos_t)
        # k[.., :HALF]  = u2 * (-sin1)   (sign folded into sin input)
        nc.vector.tensor_mul(
            k_t[:, h, :, 0:HALF], u_t[:, :, HALF:D], sin_t[:, :, 0:HALF]
        )
        # k[.., HALF:] = u1 * sin2
        nc.vector.tensor_mul(
            k_t[:, h, :, HALF:D], u_t[:, :, 0:HALF], sin_t[:, :, HALF:D]
        )
        # k += t
        nc.vector.tensor_add(k_t[:, h], k_t[:, h], t_t[:, h])

    # --- bulk copies (DRAM->DRAM), flat 1D APs, split across BOTH HWDGE
    # rings: keys+v on SP, values on ACT.  Two rings move ~11% more than
    # one (179 vs 159 GB/s measured for 4x7MiB).  The k stores go LAST on
    # ACT so its sequencer never stalls on the DVE wait mid-bulk.
    def flat_copy(eng, kv, h, dst_lo, src, src_lo, npos):
        # chunk only large runs; sub-4MiB transfers lose to per-DMA overhead
        if npos % CHUNK == 0 and npos >= 4 * SEG:
            step = npos // CHUNK
        else:
            step = npos
        nch = npos // step
        for c in range(nch):
            eng.dma_start(
                out=out[kv, h, dst_lo + c * step:dst_lo + (c + 1) * step, :]
                    .rearrange("a b -> (a b)"),
                in_=src[h, src_lo + c * step:src_lo + (c + 1) * step, :]
                    .rearrange("a b -> (a b)"),
            )

    for h in range(HPC):
        # new value segment into slot ws (pure copy)
        flat_copy(nc.sync, 1, h, ws * SEG, v, 0, SEG)
        if full_shift:
            flat_copy(nc.sync, 0, h, 0, keys, SEG, TOTAL - SEG)
            flat_copy(nc.scalar, 1, h, 0, values, SEG, TOTAL - SEG)
        else:
            if ws > 0:
                flat_copy(nc.sync, 0, h, 0, keys, 0, ws * SEG)
                flat_copy(nc.scalar, 1, h, 0, values, 0, ws * SEG)
            if ws < MEM - 1:
                flat_copy(nc.sync, 0, h, (ws + 1) * SEG, keys, (ws + 1) * SEG,
                          TOTAL - (ws + 1) * SEG)
                flat_copy(nc.scalar, 1, h, (ws + 1) * SEG, values, (ws + 1) * SEG,
                          TOTAL - (ws + 1) * SEG)

    # per-head k stores, last on the ACT ring (a combined 4-free-dim AP
    # can't be balanced, hence per-head)
    for h in range(HPC):
        nc.scalar.dma_start(
            out=out[0, h, ws * SEG:(ws + 1) * SEG, :].rearrange(
                "(n p) d -> p n d", p=PB
            ),
            in_=k_t[:, h],
        )


def _build_program(write_seg: int, full_shift: bool):
    """Build the per-core Bass program.

    write_seg: segment index where the new K/V segment lands.
    full_shift: True -> shift everything left one segment first;
                False -> copy all segments except write_seg unchanged.
    """
    import concourse.bass as bass
    import concourse.tile as tile
    from concourse import mybir

    f32 = mybir.dt.float32
    nc = bass.Bass(trn_type="TRN2", name="scatter_memory")

    keys = nc.dram_tensor("keys", [HPC, TOTAL, D], f32, kind="ExternalInput")
    values = nc.dram_tensor("values", [HPC, TOTAL, D], f32, kind="ExternalInput")
    # rope_in rows: [u(head 0), u(head 1), cos, sin'], sin' has its first
    # half negated so RoPE is mul/mul/add with no sign handling on-device.
    rope_in = nc.dram_tensor("rope_in", [RJ, SEG, D], f32, kind="ExternalInput")
    v = nc.dram_tensor("v", [HPC, SEG, D], f32, kind="ExternalInput")
    out = nc.dram_tensor("out", [2, HPC, TOTAL, D], f32, kind="ExternalOutput")

    with tile.TileContext(nc) as tc:
        with (
            tc.tile_pool(name="const", bufs=1) as const_pool,
            tc.tile_pool(name="work", bufs=2) as work_pool,
        ):
            emit_body(nc, const_pool, work_pool, rope_in, keys, values, v, out,
                      write_seg, full_shift)
    _split_multi_waits(nc, mybir)
    return nc


# Results of the most recent device run (for the test harness to inspect).
LAST_RESULTS = None


def _pack_rope(un_rotated_k_core, cos_seg, sin_mod):
    """[u(h0), u(h1), cos, sin'] -> [RJ, SEG, D] float32 contiguous."""
    packed = np.empty((RJ, SEG, D), dtype=np.float32)
    packed[:HPC] = un_rotated_k_core
    packed[HPC] = cos_seg
    packed[HPC + 1] = sin_mod
    return packed


def kernel(keys, values, un_rotated_k, v, cos_cache, sin_cache,
           position_ids, current_memory):
    from concourse.bass_utils import run_bass_kernel_spmd

    global LAST_RESULTS

    keys = np.asarray(keys, dtype=np.float32)
    values = np.asarray(values, dtype=np.float32)
    un_rotated_k = np.asarray(un_rotated_k, dtype=np.float32)
    v = np.asarray(v, dtype=np.float32)
    cos_cache = np.asarray(cos_cache, dtype=np.float32)
    sin_cache = np.asarray(sin_cache, dtype=np.float32)
    position_ids = np.asarray(position_ids)
    cm = int(current_memory)

    assert keys.shape == (B, H, TOTAL, D), keys.shape
    assert un_rotated_k.shape == (B, H, SEG, D), un_rotated_k.shape

    # Gather the RoPE tables for this segment's positions and fold the
    # rotate_half sign into sin (first half negated).
    pos = position_ids.reshape(-1)
    cos_seg = cos_cache[pos]
    sin_seg = sin_cache[pos]
    sin_mod = np.concatenate([-sin_seg[:, :HALF], sin_seg[:, HALF:]], axis=1)

    full_shift = cm >= MEM
    write_seg = MEM - 1 if full_shift else cm
    key = (write_seg, full_shift)
    if key not in _prog_cache:
        _prog_cache[key] = _build_program(write_seg, full_shift)
    nc = _prog_cache[key]

    in_maps = []
    for c in range(N_CORES):
        h0 = c * HPC
        in_maps.append({
            "keys": np.ascontiguousarray(keys[0, h0:h0 + HPC]),
            "values": np.ascontiguousarray(values[0, h0:h0 + HPC]),
            "rope_in": _pack_rope(un_rotated_k[0, h0:h0 + HPC], cos_seg, sin_mod),
            "v": np.ascontiguousarray(v[0, h0:h0 + HPC]),
        })

    res = run_bass_kernel_spmd(nc, in_maps, core_ids=list(range(N_CORES)))
    LAST_RESULTS = res

    full = np.empty((2, B, H, TOTAL, D), dtype=np.float32)
    for c in range(N_CORES):
        h0 = c * HPC
        full[0, 0, h0:h0 + HPC] = res.results[c]["out"][0]
        full[1, 0, h0:h0 + HPC] = res.results[c]["out"][1]
    return full
